# revision 36
# baseline (speedup 1.0000x reference)
"""Epipolar attention kernel for Trainium2 (8 NeuronCores, batch-parallel).

Math notes (derived from the reference):
  - f_tar is dead code: the output only depends on f_src / K1 / K2 / R / t.
  - With x0=0, x1=W the distance field factorizes rank-3:
        d[b,i,j] = |px_i*alpha[b,j] + py_i*beta[b,j] + gamma[b,j]|
    where alpha = dy/L, beta = -dx/L, gamma = y0*dx/L, L = sqrt(dx^2+dy^2).
  - softmax_j(5*(d-0.1)) == softmax_j(5*d)           (shift invariance)
  - softmax_i(1 - p)     == softmax_i(-p), and p in (0,1] means exp(-p) needs
    no max subtraction.
The 3x3 SVD / inverse chain (B=16) is O(B) host work; all O(B*HW^2) work runs
on the NeuronCores.

v2 engine split (per core = 2 batches):
  - PE:  S = P^T Q in fp32r (1 cyc/row, no hi/lo split), e-transpose via
         diag(1/s1) matmul, final GEMM in bf16.
  - DVE: fused |S| + row-max in ONE tensor_tensor_reduce pass:
         zneg = (S abs_max 0) * -5, accum_out = min -> -m  (the exp1 bias).
         Plus reciprocals, diag scale, f row scale.
  - ACT: only the two exp passes (exp1 with accum s1, exp2 with accum s2)
         plus PSUM->SBUF bf16 evictions of the GEMM output.
  - Output is bf16 (upcast on host) to halve the store DMA.
"""

import numpy as np
import ml_dtypes

import concourse.bass as bass
import concourse.bacc as bacc
import concourse.tile as tile
import concourse.mybir as mybir
from concourse.bass_utils import run_bass_kernel_spmd
from concourse import dve_ops as _dve_ops
from concourse.dve_spec import Spec as _Spec, Src0 as _Src0, C0 as _C0, C1 as _C1, \
    Zero as _Zero, minn as _minn


def _register_absmin_op():
    """Custom DVE op (documented plugin path: append to dve_ops.OPS):
        out      = min(in0*s0, in0*s1)            # s0=-5, s1=+5 -> -5*|in0|
        accum_out= min(0, min_k out[k])           # = -5*max|in0| = -m
    Fuses the |S| materialization, the *5 scale and the row-max reduction
    into ONE Vector-engine pass over the PSUM S tile.
    """
    name = "ABS_NEG_MINREDUCE_ANT"
    for op in _dve_ops.OPS:
        if op.name == name:
            return op

    def _ref(in0, in1, c0, c1, c2):
        b = np.minimum(
            in0.astype(np.float32) * c0, in0.astype(np.float32) * c1
        ).astype(np.float32)
        return b, np.minimum(0.0, b.reshape(b.shape[0], -1).min(-1, keepdims=True))

    op = _dve_ops.DveOp(
        name,
        _Spec(
            body=_minn(_Src0 * _C0, _Src0 * _C1),
            accum=_minn,
            accum_init=_Zero,
            reference=_ref,
        ),
        subdim=False,
        uops_sha={"v3": "99f7aa2da1e18e77", "v4": "ed7bfe85cf3d1c3c"},
    )
    _dve_ops.OPS.append(op)
    _dve_ops.CUSTOM_DVE_SPECS[name] = op.spec
    _dve_ops._SUB_OPCODE_FOR_NAME[name] = (
        _dve_ops._CUSTOM_DVE_ROW_BASE + len(_dve_ops.OPS) - 1
    )
    return op


_ABSOP = _register_absmin_op()

B, C, H, W = 16, 512, 32, 32
HW = H * W          # 1024
NCORES = 8
BPC = B // NCORES   # batches per core
NT = HW // 128      # 128-row tiles per HW dim
F32 = mybir.dt.float32
F32R = mybir.dt.float32r
BF16 = mybir.dt.bfloat16
AF = mybir.ActivationFunctionType
AX = mybir.AxisListType
ALU = mybir.AluOpType


# ---------------------------------------------------------------- host math
def _line_coeffs(K1, K2, R, t):
    """Float32 numpy mirror of the reference's per-batch line geometry.

    Returns Q (B, 3, HW) with rows [alpha, beta, gamma] and P (3, HW) with
    rows [px, py, 1].
    """
    K1 = np.asarray(K1, np.float32)
    K2 = np.asarray(K2, np.float32)
    R = np.asarray(R, np.float32)
    t = np.asarray(t, np.float32)

    z = np.zeros_like(t[:, 0])
    tx, ty, tz = t[:, 0], t[:, 1], t[:, 2]
    skew = np.stack(
        [
            np.stack([z, -tz, ty], axis=-1),
            np.stack([tz, z, -tx], axis=-1),
            np.stack([-ty, tx, z], axis=-1),
        ],
        axis=1,
    )
    E = skew @ R
    U, S, Vt = np.linalg.svd(E)
    S = S * np.array([1.0, 1.0, 0.0], dtype=S.dtype)
    E = U @ (S[:, :, None] * Vt)
    Fm = np.linalg.inv(np.swapaxes(K2, 1, 2)) @ E @ np.linalg.inv(K1)
    Fm = Fm.astype(np.float32)

    ix, iy = np.meshgrid(
        np.arange(H, dtype=np.float32), np.arange(W, dtype=np.float32), indexing="ij"
    )
    px = ix.reshape(-1)
    py = iy.reshape(-1)
    idx = np.stack([px, py, np.ones_like(px)], axis=0)  # (3, HW)

    lines = Fm @ idx[None]  # (B, 3, HW)
    a, b, c = lines[:, 0], lines[:, 1], lines[:, 2]
    x0 = np.zeros_like(a)
    y0 = -c / b
    x1 = np.full_like(a, float(W))
    y1 = -(c + a * float(W)) / b
    dx = x0 - x1
    dy = y0 - y1
    L = np.sqrt(dx * dx + dy * dy)

    alpha = dy / L
    beta = -dx / L
    gamma = (y0 * dx) / L
    Q = np.stack([alpha, beta, gamma], axis=1).astype(np.float32)  # (B, 3, HW)
    P = idx.astype(np.float32)
    return Q, P


# ---------------------------------------------------------------- device IR
def _build_nc():
    nc = bacc.Bacc("TRN2", target_bir_lowering=False, debug=False)

    pmat_d = nc.dram_tensor("pmat", [3, HW], F32R, kind="ExternalInput")
    qmat_d = nc.dram_tensor("qmat", [BPC, 3, HW], F32R, kind="ExternalInput")
    fsrc_d = nc.dram_tensor("fsrc", [BPC, HW, C], BF16, kind="ExternalInput")
    ident_d = nc.dram_tensor("ident", [128, 128], BF16, kind="ExternalInput")
    out_d = nc.dram_tensor("out", [BPC, HW, C], BF16, kind="ExternalOutput")

    with tile.TileContext(nc) as tc:
        with (
            tc.tile_pool(name="const", bufs=1) as const,
            tc.tile_pool(name="q", bufs=2) as qpool,
            tc.tile_pool(name="f", bufs=2) as fpool,
            tc.tile_pool(name="z", bufs=8) as zpool,
            tc.tile_pool(name="e", bufs=2) as epool,
            tc.tile_pool(name="dg", bufs=2) as dgpool,
            tc.tile_pool(name="e2", bufs=2) as e2pool,
            tc.tile_pool(name="stat", bufs=2) as stat,
            tc.tile_pool(name="o", bufs=4) as opool,
            tc.tile_pool(name="sps", bufs=2, space="PSUM") as spspool,
            tc.tile_pool(name="ps", bufs=2, space="PSUM") as pspool,
        ):
            pm = const.tile([3, HW], F32R)
            nc.sync.dma_start(pm[:], pmat_d[:])
            idn = const.tile([128, 128], BF16)
            nc.sync.dma_start(idn[:], ident_d[:])

            st = [dict() for _ in range(BPC)]

            def load(b):
                s = st[b]
                s["q"] = qpool.tile([3, HW], F32R, tag="q", name="q")
                nc.sync.dma_start(s["q"][:], qmat_d[b])
                s["fa"] = fpool.tile([128, NT, C], BF16, tag="fa", name="fa")
                for tj in range(NT):
                    nc.sync.dma_start(
                        s["fa"][:, tj, :], fsrc_d[b, tj * 128 : (tj + 1) * 128, :]
                    )
                s["ea"] = epool.tile([128, NT, HW], BF16, tag="ea", name="ea")
                s["nm"] = stat.tile([128, NT], F32, tag="nm", name="nm")
                s["s1"] = stat.tile([128, NT], F32, tag="s1", name="s1")
                s["r1"] = stat.tile([128, NT], F32, tag="r1", name="r1")
                s["dga"] = dgpool.tile([128, NT, 128], BF16, tag="dga", name="dga")
                s["e2"] = e2pool.tile([128, NT, HW], BF16, tag="e2", name="e2")
                s["s2"] = stat.tile([128, NT], F32, tag="s2", name="s2")
                s["r2"] = stat.tile([128, NT], F32, tag="r2", name="r2")

            def stage1_mm(b, ti, pool=None):
                # S = P^T Q in fp32r (full-precision, 1 cyc/row at N>=256)
                s = st[b]
                pl = pool or spspool
                sp = pl.tile(
                    [128, HW], F32, tag=("sp" if pl is spspool else "ps"), name="sp"
                )
                s[("sp", ti)] = sp
                pmr = pm[:, ti * 128 : (ti + 1) * 128]
                for nh in range(2):
                    nc.tensor.matmul(
                        sp[:, nh * 512 : (nh + 1) * 512],
                        pmr,
                        s["q"][:, nh * 512 : (nh + 1) * 512],
                        start=True,
                        stop=True,
                    )

            def stage1_abs(b, ti):
                # Fused DVE pass: zneg = -5*|S| (SBUF f32) and row -max via
                # min-accum -> nm = -m, directly the exp1 bias.
                s = st[b]
                sp = s.pop(("sp", ti))
                zt = zpool.tile([128, HW], F32)
                s[("zt", ti)] = zt
                nc.vector._custom_dve(
                    _ABSOP,
                    out=zt[:],
                    in0=sp[:],
                    s0=-5.0,
                    s1=5.0,
                    accum_out=s["nm"][:, ti : ti + 1],
                )

            def stage1_exp(b, ti, dga_eng="pool"):
                # e = exp(-zneg + nm) = exp(5|S| - m), accum s1; then r1 and
                # the scaled identity.
                s = st[b]
                zt = s.pop(("zt", ti))
                nc.scalar.activation(
                    s["ea"][:, ti, :],
                    zt[:],
                    AF.Exp,
                    bias=s["nm"][:, ti : ti + 1],
                    scale=-1.0,
                    accum_out=s["s1"][:, ti : ti + 1],
                )
                nc.vector.reciprocal_approx_fast(
                    s["r1"][:, ti : ti + 1], s["s1"][:, ti : ti + 1]
                )
                eng = nc.gpsimd if dga_eng == "pool" else nc.vector
                eng.tensor_scalar_mul(
                    s["dga"][:, ti, :], idn[:], s["r1"][:, ti : ti + 1]
                )

            def stage1(b, ti):
                stage1_mm(b, ti)
                stage1_abs(b, ti)
                stage1_exp(b, ti, dga_eng="vec")

            def stage2_mm(b, tj, tis=(0, 4, 1, 5, 2, 6, 3, 7)):
                # "transpose" via real matmul: PT[j,i'] = sum_i e[i,j]*dga[i,i']
                # = e[i',j]/s1[i'].  Bank-alternating ti order so consecutive
                # writes don't serialize on the PSUM bank tracker.
                s = st[b]
                key = ("tp", tj)
                if key not in s:
                    s[key] = pspool.tile([128, HW], F32, tag="ps", name="tp")
                tp = s[key]
                for ti in tis:
                    nc.tensor.matmul(
                        tp[:, ti * 128 : (ti + 1) * 128],
                        s["ea"][:, ti, tj * 128 : (tj + 1) * 128],
                        s["dga"][:, ti, :],
                        start=True,
                        stop=True,
                    )

            def stage2_post(b, tj):
                # E2 = exp(-p) with column sums; fold 1/s2 into the f rows.
                s = st[b]
                tp = s.pop(("tp", tj))
                nc.scalar.activation(
                    s["e2"][:, tj, :],
                    tp[:],
                    AF.Exp,
                    scale=-1.0,
                    accum_out=s["s2"][:, tj : tj + 1],
                )
                nc.vector.reciprocal_approx_fast(
                    s["r2"][:, tj : tj + 1], s["s2"][:, tj : tj + 1]
                )
                nc.vector.tensor_scalar_mul(
                    s["fa"][:, tj, :], s["fa"][:, tj, :], s["r2"][:, tj : tj + 1]
                )

            def stage3_mm(b, tg, tjs=tuple(range(NT)), pool=None):
                # GEMM: out[i, c] = sum_j exp(-p)[j,i] * fw[j, c]
                # Two i-tiles per 2-bank PSUM slot (reuses the stage-1 pool).
                s = st[b]
                key = ("op", tg)
                if key not in s:
                    pl = pool or spspool
                    s[key] = pl.tile(
                        [128, 2, C], F32,
                        tag=("sp" if pl is spspool else "ps"), name="op",
                    )
                op_ = s[key]
                for half in range(2):
                    ti = 2 * tg + half
                    for tj in tjs:
                        nc.tensor.matmul(
                            op_[:, half, :],
                            s["e2"][:, tj, ti * 128 : (ti + 1) * 128],
                            s["fa"][:, tj, :],
                            start=(tj == 0),
                            stop=(tj == NT - 1),
                        )

            def stage3_out(b, tg, eng="act"):
                s = st[b]
                op_ = s.pop(("op", tg))
                ost = opool.tile([128, 2, C], BF16)
                if eng == "act":
                    nc.scalar.copy(ost[:], op_[:])
                elif eng == "split":
                    # halve the eviction latency: ACT and DVE each take half
                    nc.scalar.copy(ost[:, 0, :], op_[:, 0, :])
                    nc.vector.tensor_copy(ost[:, 1, :], op_[:, 1, :])
                else:
                    nc.vector.tensor_copy(ost[:], op_[:])
                nc.sync.dma_start(
                    out_d[b, tg * 256 : (tg + 1) * 256, :].rearrange(
                        "(t p) c -> p t c", p=128
                    ),
                    ost[:],
                )

            # ---- schedule -------------------------------------------------
            # Engine FIFO targets (per-engine order = emission order):
            #  ACT: exp1(0)x8 | exp2(0)x8 | exp1(1)x8 | exp2(1)x8 | b1 evicts
            #  PE : S(0) | [transp(0,k), S(1,k), G0-incr(k-2)] | G0 g2,g3 |
            #       transp(1,0-1) | [transp(1,k+2), G1-incr(k), G1 g2/g3 late]
            #  DVE: stage1(0) | [recip2/fa(0,k), abs(1,k)] | ev(0,g0/g1) |
            #       recip1/dga(1) | ev(0,g2/g3) | [recip2/fa(1,k)]
            #  Pool: dga(0) scaling only (GPSIMD cannot touch PSUM).
            # sp(1) tiles share the transpose pool's two slots (tag "ps") so
            # the "sp" slots are free for the incremental GEMM(0) groups that
            # run under ACT's exp2(0) stream.
            load(0)
            load(1)
            # phase A: stage1(0)
            for ti in range(NT):
                stage1(0, ti)
            # phase B: PE trickles transp(0,k)+S(1,k) at the exp2(0) pace;
            # GEMM(0) groups 0/1 accumulate incrementally two tiles behind.
            # abs(1,k) is emitted BEFORE recip2/fa(0,k) on DVE so the sp(1)
            # slots turn over without waiting on the ACT-gated ops.
            for k in range(NT):
                stage2_mm(0, k)
                stage1_mm(1, k, pool=pspool)
                stage1_abs(1, k)
                stage2_post(0, k)
                stage1_exp(1, k, dga_eng="vec")
                if k >= 2:
                    for g in (0, 1):
                        stage3_mm(0, g, tjs=(k - 2,))
            for k in (NT - 2, NT - 1):
                for g in (0, 1):
                    stage3_mm(0, g, tjs=(k,))
            for g in (0, 1):
                stage3_out(0, g, eng="vec")
            # GEMM(0) g2/g3 in half-group chunks interleaved with the batch-1
            # transposes so the transposes stay ahead of ACT's exp2(1) stream.
            stage2_mm(1, 0)
            stage2_mm(1, 1)
            stage3_mm(0, 2, tjs=tuple(range(4)))
            stage2_mm(1, 2)
            stage3_mm(0, 2, tjs=tuple(range(4, NT)))
            stage3_out(0, 2, eng="vec")
            stage2_mm(1, 3)
            stage3_mm(0, 3, tjs=tuple(range(4)))
            stage2_mm(1, 4)
            stage3_mm(0, 3, tjs=tuple(range(4, NT)))
            stage3_out(0, 3, eng="vec")
            for k in (5, 6, 7):
                stage2_mm(1, k)
            # phase D: exp2(1) stream on ACT; GEMM(1) groups catch up
            # incrementally as the "sp" slots free and e2(1) tiles land;
            # group 2 starts when a transpose-pool slot frees.
            for k in range(NT):
                stage2_post(1, k)
                if k >= 4:
                    stage3_mm(1, 0, tjs=tuple(range(5)) if k == 4 else (k,))
                if k >= 5:
                    stage3_mm(1, 1, tjs=tuple(range(6)) if k == 5 else (k,))
                if k == 6:
                    stage3_mm(1, 2, tjs=tuple(range(7)), pool=pspool)
            stage3_mm(1, 2, tjs=(NT - 1,))
            for g in (0, 1, 2):
                stage3_out(1, g, eng="split")
            stage3_mm(1, 3, pool=pspool)
            stage3_out(1, 3, eng="split")
    nc.compile()
    return nc


_NC = None


def _get_nc():
    global _NC
    if _NC is None:
        _NC = _build_nc()
    return _NC


# ---------------------------------------------------------------- execution
def _run(inputs, trace=False):
    f_src = np.asarray(inputs["f_src"], np.float32)
    Q, P = _line_coeffs(inputs["K1"], inputs["K2"], inputs["R"], inputs["t"])

    fsrcT = np.ascontiguousarray(
        f_src.reshape(B, C, HW).transpose(0, 2, 1)
    ).astype(ml_dtypes.bfloat16)
    ident = np.eye(128, dtype=np.float32).astype(ml_dtypes.bfloat16)

    in_maps = []
    for core in range(NCORES):
        lo = core * BPC
        hi = lo + BPC
        in_maps.append(
            {
                "pmat": P,
                "qmat": np.ascontiguousarray(Q[lo:hi]),
                "fsrc": np.ascontiguousarray(fsrcT[lo:hi]),
                "ident": ident,
            }
        )

    nc = _get_nc()
    res = run_bass_kernel_spmd(nc, in_maps, list(range(NCORES)), trace=trace)
    out_flat = np.concatenate(
        [np.asarray(res.results[i]["out"]) for i in range(NCORES)], axis=0
    )  # (B, HW, C) bf16
    out = np.ascontiguousarray(out_flat.astype(np.float32)).reshape(B, C, H, W)
    return out, res


def kernel(**inputs):
    out, _ = _run(inputs, trace=False)
    return out
